# revision 16
# baseline (speedup 1.0000x reference)
"""Trainium2 Bass kernel for nn_AudioLiquidEmberBlock.

Computation (per batch element, B=8 sharded across 8 NeuronCores):
  normed = LN1(x)                                      # [T=1024, D=1024]
  scan over T with CfC cell (tanh/tanh/sigmoid gates, U=1536)
  fused with LIF spiking gate on the first D hidden channels
  x = x + gated; x = x + MLP(LN2(x))                   # MLP d_ff=4096
  returns (x, h_final)

Strategy:
  - Batch-parallel over 8 cores (batch 8 -> 1 sequence per core).
  - The x-contribution of the CfC gates (x_t @ W_x + b) is hoisted out of
    the scan into one big GEMM; only the h-recurrence (h @ W_h) runs in the
    sequential loop. Since TS == 1.0, the ta/tb gates fold into a single
    combined gate matrix, so the scan does 3 gate matmuls, not 4.
  - Scan uses gate-major layout: weights stationary on the PE
    ([128,128] bf16 tiles, FWL), hidden state streams as rhs columns.
  - All matmuls bf16 with fp32 PSUM accumulation; LN/elementwise fp32.
"""
import os
import sys

sys.path.insert(0, '/opt/trn_rl_repo')

import numpy as np
import ml_dtypes

import concourse.bass as bass
import concourse.tile as tile
from concourse import bacc, mybir
from concourse.bass import ds
from concourse.bass_utils import run_bass_kernel_spmd

P = 128
D = 1024          # n_embd
U = 1536          # CfC units
DFF = 4096        # MLP hidden
T = 1024          # sequence length
B = 8             # batch == number of cores
KT = U // P       # 12 k-tiles for the recurrence
MT = 3 * (U // P)  # 36 m-tiles (3 gates after ta/tb fold)
DT = D // P       # 8
FT = DFF // P     # 32
UNROLL = 16
HALF = KT // 2
NCH = 256         # T-chunk for the xw precompute GEMM
EPS = 1e-5
SLOPE = 5.0
TS = 1.0

f32 = mybir.dt.float32
bf16 = mybir.dt.bfloat16
AF = mybir.ActivationFunctionType
ET = mybir.EngineType

TRACE = bool(int(os.environ.get("KERNEL_TRACE", "0")))
last_exec_time_ns = None

_prog_cache = {}


def _layer_norm(nc, pool, x_sb, g_bc, b_bc, out_sb, tag):
    """LN over the free dim (size D) of x_sb [P, D] fp32 -> out_sb."""
    mu = pool.tile([P, 1], f32, tag=f"{tag}_mu", name=f"{tag}_mu")
    nc.vector.reduce_sum(mu[:], x_sb[:], axis=mybir.AxisListType.X)
    nc.vector.tensor_scalar_mul(mu[:], mu[:], 1.0 / D)
    xc = pool.tile([P, D], f32, tag=f"{tag}_xc", name=f"{tag}_xc")
    nc.vector.tensor_scalar_sub(xc[:], x_sb[:], mu[:])
    sq = pool.tile([P, D], f32, tag=f"{tag}_sq", name=f"{tag}_sq")
    var = pool.tile([P, 1], f32, tag=f"{tag}_var", name=f"{tag}_var")
    nc.scalar.activation(sq[:], xc[:], AF.Square, accum_out=var[:])
    rstd = pool.tile([P, 1], f32, tag=f"{tag}_rstd", name=f"{tag}_rstd")
    nc.vector.tensor_scalar(
        out=rstd[:], in0=var[:], scalar1=1.0 / D, scalar2=EPS,
        op0=mybir.AluOpType.mult, op1=mybir.AluOpType.add)
    nc.scalar.activation(rstd[:], rstd[:], AF.Sqrt)
    nc.vector.reciprocal(rstd[:], rstd[:])
    nc.vector.tensor_scalar_mul(xc[:], xc[:], rstd[:])
    nc.vector.tensor_mul(xc[:], xc[:], g_bc)
    nc.vector.tensor_add(out_sb[:], xc[:], b_bc)


def _build_program():
    nc = bacc.Bacc("TRN2", target_bir_lowering=False, debug=False,
                   enable_asserts=False, num_devices=B)

    x_d = nc.dram_tensor("x", [T, D], f32, kind="ExternalInput")
    hx_d = nc.dram_tensor("hx", [P, KT], f32, kind="ExternalInput")
    id_d = nc.dram_tensor("ident", [P, P], bf16, kind="ExternalInput")
    wh_d = nc.dram_tensor("wh", [P, KT, MT, P], bf16, kind="ExternalInput")
    wx_d = nc.dram_tensor("wx", [P, DT, MT, P], bf16, kind="ExternalInput")
    bias_d = nc.dram_tensor("bias", [P, MT], f32, kind="ExternalInput")
    beta_d = nc.dram_tensor("beta", [P, DT], f32, kind="ExternalInput")
    vth_d = nc.dram_tensor("vth", [P, DT], f32, kind="ExternalInput")
    ln1g_d = nc.dram_tensor("ln1g", [D], f32, kind="ExternalInput")
    ln1b_d = nc.dram_tensor("ln1b", [D], f32, kind="ExternalInput")
    ln2g_d = nc.dram_tensor("ln2g", [D], f32, kind="ExternalInput")
    ln2b_d = nc.dram_tensor("ln2b", [D], f32, kind="ExternalInput")
    w1_d = nc.dram_tensor("w1", [P, DT, FT, P], bf16, kind="ExternalInput")
    b1_d = nc.dram_tensor("b1", [P, FT], f32, kind="ExternalInput")
    w2_d = nc.dram_tensor("w2", [P, FT, DT, P], bf16, kind="ExternalInput")
    b2_d = nc.dram_tensor("b2", [P, DT], f32, kind="ExternalInput")

    y_out = nc.dram_tensor("y_out", [T, D], f32, kind="ExternalOutput")
    h_out = nc.dram_tensor("h_out", [P, KT], f32, kind="ExternalOutput")

    # DRAM scratch
    nrm_d = nc.dram_tensor("nrm_scratch", [T, D], bf16)
    xw_d = nc.dram_tensor("xw_scratch", [P, T, MT], bf16)
    gd_d = nc.dram_tensor("gd_scratch", [D, T], bf16)
    h2_d = nc.dram_tensor("h2_scratch", [T, D], bf16)
    o2_d = nc.dram_tensor("o2_scratch", [D, T], bf16)

    with tile.TileContext(nc) as tc:
        if True:
            # ---------------- Phase 1: LN1, write normed (bf16) ----------
            with (tc.tile_pool(name="ln1", bufs=2) as lp,
                  tc.tile_pool(name="lnc", bufs=1) as lc):
                ln1g_sb = lc.tile([P, D], f32, name="ln1g_sb")
                nc.sync.dma_start(ln1g_sb[:], ln1g_d[None, :].to_broadcast([P, D]))
                ln1b_sb = lc.tile([P, D], f32, name="ln1b_sb")
                nc.sync.dma_start(ln1b_sb[:], ln1b_d[None, :].to_broadcast([P, D]))
                g_bc = ln1g_sb[:]
                b_bc = ln1b_sb[:]
                for tt in range(DT):
                    xt = lp.tile([P, D], f32, tag="xt", name="xt")
                    nc.sync.dma_start(xt[:], x_d[tt * P:(tt + 1) * P, :])
                    nrm = lp.tile([P, D], f32, tag="nrm", name="nrm")
                    _layer_norm(nc, lp, xt, g_bc, b_bc, nrm, "ln1")
                    nrmb = lp.tile([P, D], bf16, tag="nrmb", name="nrmb")
                    nc.vector.tensor_copy(nrmb[:], nrm[:])
                    nc.sync.dma_start(nrm_d[tt * P:(tt + 1) * P, :], nrmb[:])

            # ---------------- Phase 2+3: xw = normed @ Wx + b ------------
            with (tc.tile_pool(name="gm1w", bufs=1) as gw,
                  tc.tile_pool(name="gm1s", bufs=2) as gs,
                  tc.tile_pool(name="gm1p", bufs=4, space="PSUM") as gp):
                nrmT_sb = gw.tile([P, DT, T], bf16, name="nrmT_sb")  # 2 MB
                for dt_ in range(DT):
                    nc.sync.dma_start_transpose(
                        nrmT_sb[:, dt_, :], nrm_d[:, dt_ * P:(dt_ + 1) * P])
                wx_sb = gw.tile([P, DT, MT, P], bf16, name="wx_sb")  # 9.4 MB
                nc.sync.dma_start(wx_sb[:], wx_d[:])
                bias_sb = gw.tile([P, MT], f32, name="bias_sb")
                nc.sync.dma_start(bias_sb[:], bias_d[:])
                for n in range(T // NCH):
                    stg = gs.tile([P, NCH, MT], bf16, tag="stg", name="stg")
                    for m in range(MT):
                        psA = gp.tile([P, NCH], f32, tag="psA", name="psA")
                        for k in range(DT):
                            nc.tensor.matmul(
                                psA[:], wx_sb[:, k, m, :],
                                nrmT_sb[:, k, n * NCH:(n + 1) * NCH],
                                start=(k == 0), stop=(k == DT - 1))
                        nc.scalar.activation(stg[:, :, m], psA[:], AF.Identity,
                                             bias=bias_sb[:, m:m + 1])
                    nc.sync.dma_start(xw_d[:, n * NCH:(n + 1) * NCH, :], stg[:])

            # ---------------- Phase 4: the scan --------------------------
            with (tc.tile_pool(name="scw", bufs=1) as sw,
                  tc.tile_pool(name="scs", bufs=1) as st,
                  tc.tile_pool(name="scx", bufs=3) as sx,
                  tc.tile_pool(name="scwk", bufs=3) as wk,
                  tc.tile_pool(name="scps", bufs=2, space="PSUM") as pp):
                wh_sb = sw.tile([P, KT, MT, P], bf16, name="wh_sb")  # 14.2 MB
                nc.sync.dma_start(wh_sb[:], wh_d[:])
                id_sb = sw.tile([P, P], bf16, name="id_sb")
                nc.sync.dma_start(id_sb[:], id_d[:])
                beta_sb = st.tile([P, DT], f32, name="beta_sb")
                nc.sync.dma_start(beta_sb[:], beta_d[:])
                nc.scalar.activation(beta_sb[:], beta_sb[:], AF.Sigmoid)
                vth_sb = st.tile([P, DT], f32, name="vth_sb")
                nc.sync.dma_start(vth_sb[:], vth_d[:])
                gatedT = st.tile([P, DT, T], bf16, name="gatedT")   # 2 MB
                mem = st.tile([P, DT], f32, name="mem")
                nc.any.memset(mem[:], 0.0)
                hT = []
                for i in range(4):
                    ha = st.tile([P, HALF], bf16, name=f"h{i}a")
                    hb = st.tile([P, HALF], bf16, name=f"h{i}b")
                    hT.append((ha, hb))
                hf32 = st.tile([P, KT], f32, name="hf32")
                hx_sb = st.tile([P, KT], f32, name="hx_sb")
                nc.sync.dma_start(hx_sb[:], hx_d[:])
                nc.vector.tensor_copy(hT[0][0][:], hx_sb[:, 0:HALF])
                nc.vector.tensor_copy(hT[0][1][:], hx_sb[:, HALF:KT])
                nc.vector.tensor_copy(hf32[:], hx_sb[:])

                with tc.For_i(0, T, UNROLL, hint_engines=(ET.PE,),
                              staggered_reset=True) as t0:
                    xwc = sx.tile([P, UNROLL, MT], bf16, name="xwc")
                    nc.sync.dma_start(xwc[:], xw_d[:, ds(t0, UNROLL), :])

                    def init_psums(u):
                        pst = pp.tile([P, KT], f32, tag="pst", name="pst",
                                      padded_shape=[P, 512])
                        ps2 = pp.tile([P, KT], f32, tag="ps2", name="ps2",
                                      padded_shape=[P, 512])
                        ps1a = pp.tile([P, HALF], f32, tag="ps1a", name="ps1a",
                                       padded_shape=[P, 512])
                        ps1b = pp.tile([P, HALF], f32, tag="ps1b", name="ps1b",
                                       padded_shape=[P, 512])
                        nc.tensor.matmul(pst[:], id_sb[:],
                                         xwc[:, u, 2 * KT:3 * KT],
                                         start=True, stop=False)
                        nc.tensor.matmul(ps2[:], id_sb[:],
                                         xwc[:, u, KT:2 * KT],
                                         start=True, stop=False)
                        nc.tensor.matmul(ps1a[:], id_sb[:], xwc[:, u, 0:HALF],
                                         start=True, stop=False)
                        nc.tensor.matmul(ps1b[:], id_sb[:], xwc[:, u, HALF:KT],
                                         start=True, stop=False)
                        return pst, ps2, ps1a, ps1b

                    psums = init_psums(0)
                    for u in range(UNROLL):
                        curA, curB = hT[u % 4]
                        nxtA, nxtB = hT[(u + 1) % 4]

                        def rhs_col(k):
                            return curA[:, k:k + 1] if k < HALF else \
                                curB[:, k - HALF:k - HALF + 1]

                        pst, ps2, ps1a, ps1b = psums
                        # tc gate: K-OUTER (consumes cur column-by-column)
                        for k in range(KT):
                            for mi in range(KT):
                                nc.tensor.matmul(
                                    pst[:, mi:mi + 1],
                                    wh_sb[:, k, 2 * KT + mi, :],
                                    rhs_col(k), start=False,
                                    stop=(k == KT - 1), skip_group_check=True)
                        sg = wk.tile([P, KT], f32, tag="sg", name="sg")
                        nc.scalar.activation(sg[:], pst[:], AF.Sigmoid)
                        oms = wk.tile([P, KT], f32, tag="oms", name="oms")
                        nc.vector.tensor_scalar(
                            out=oms[:], in0=sg[:], scalar1=-1.0, scalar2=1.0,
                            op0=mybir.AluOpType.mult, op1=mybir.AluOpType.add)
                        if u + 1 < UNROLL:
                            psums = init_psums(u + 1)
                        # ff2 gate: m-outer
                        for mi in range(KT):
                            for k in range(KT):
                                nc.tensor.matmul(
                                    ps2[:, mi:mi + 1], wh_sb[:, k, KT + mi, :],
                                    rhs_col(k), start=False,
                                    stop=(k == KT - 1))
                        ff2 = wk.tile([P, KT], f32, tag="ff2", name="ff2")
                        nc.scalar.activation(ff2[:], ps2[:], AF.Tanh)
                        cc = wk.tile([P, KT], f32, tag="cc", name="cc")
                        nc.vector.tensor_mul(cc[:], sg[:], ff2[:])
                        # ff1 gate: m-outer, tail in column halves
                        ff1 = wk.tile([P, KT], f32, tag="ff1", name="ff1")
                        t1 = wk.tile([P, KT], f32, tag="t1", name="t1")
                        for half, (psh, nxth) in enumerate(((ps1a, nxtA),
                                                            (ps1b, nxtB))):
                            sl = slice(half * HALF, (half + 1) * HALF)
                            for mj in range(HALF):
                                mi = half * HALF + mj
                                for k in range(KT):
                                    nc.tensor.matmul(
                                        psh[:, mj:mj + 1], wh_sb[:, k, mi, :],
                                        rhs_col(k), start=False,
                                        stop=(k == KT - 1))
                            nc.scalar.activation(ff1[:, sl], psh[:], AF.Tanh)
                            nc.vector.tensor_mul(t1[:, sl], ff1[:, sl],
                                                 oms[:, sl])
                            nc.vector.tensor_add(nxth[:], t1[:, sl], cc[:, sl])
                        # off path: fp32 h for LIF + h_final
                        nc.vector.tensor_add(hf32[:], t1[:], cc[:])
                        # LIF membrane update on the first DT tiles
                        m1 = wk.tile([P, DT], f32, tag="m1", name="m1")
                        nc.vector.tensor_mul(m1[:], mem[:], beta_sb[:])
                        nc.vector.tensor_add(m1[:], m1[:], hf32[:, 0:DT])
                        spin = wk.tile([P, DT], f32, tag="spin", name="spin")
                        nc.vector.tensor_sub(spin[:], m1[:], vth_sb[:])
                        spk = wk.tile([P, DT], f32, tag="spk", name="spk")
                        nc.scalar.activation(spk[:], spin[:], AF.Sigmoid,
                                             scale=SLOPE)
                        sv = wk.tile([P, DT], f32, tag="sv", name="sv")
                        nc.vector.tensor_mul(sv[:], spk[:], vth_sb[:])
                        nc.vector.tensor_sub(mem[:], m1[:], sv[:])
                        nc.vector.tensor_mul(gatedT[:, :, ds(t0 + u, 1)],
                                             hf32[:, 0:DT, None],
                                             spk[:, :, None])
                nc.sync.dma_start(h_out[:], hf32[:])
                nc.sync.dma_start(
                    gd_d.rearrange("(dt p) t -> p dt t", p=P), gatedT[:])

            # ---------------- Phase 5: residual + LN2 --------------------
            pers_cm = tc.tile_pool(name="persist", bufs=1)
            pers = pers_cm.__enter__()
            x2_sb = pers.tile([P, DT, D], f32, name="x2_sb")       # 4 MB
            h2T_sb = pers.tile([P, DT, T], bf16, name="h2T_sb")    # 2 MB
            mlp1T_sb = pers.tile([P, FT, T], bf16, name="mlp1T_sb")  # 8 MB
            with (tc.tile_pool(name="ln2", bufs=2) as lp2,
                  tc.tile_pool(name="lnc2", bufs=1) as lc2):
                ln2g_sb = lc2.tile([P, D], f32, name="ln2g_sb")
                nc.sync.dma_start(ln2g_sb[:], ln2g_d[None, :].to_broadcast([P, D]))
                ln2b_sb = lc2.tile([P, D], f32, name="ln2b_sb")
                nc.sync.dma_start(ln2b_sb[:], ln2b_d[None, :].to_broadcast([P, D]))
                g2_bc = ln2g_sb[:]
                b2_bc = ln2b_sb[:]
                for tt in range(DT):
                    gtok = lp2.tile([P, D], bf16, tag="gtok", name="gtok")
                    nc.sync.dma_start_transpose(
                        gtok[:], gd_d[:, tt * P:(tt + 1) * P])
                    xt2 = lp2.tile([P, D], f32, tag="xt2", name="xt2")
                    nc.sync.dma_start(xt2[:], x_d[tt * P:(tt + 1) * P, :])
                    nc.vector.tensor_add(x2_sb[:, tt, :], xt2[:], gtok[:])
                    h2t = lp2.tile([P, D], f32, tag="h2t", name="h2t")
                    _layer_norm(nc, lp2, x2_sb[:, tt, :], g2_bc, b2_bc, h2t,
                                "ln2")
                    h2b = lp2.tile([P, D], bf16, tag="h2b", name="h2b")
                    nc.vector.tensor_copy(h2b[:], h2t[:])
                    nc.sync.dma_start(h2_d[tt * P:(tt + 1) * P, :], h2b[:])
                for dt_ in range(DT):
                    nc.sync.dma_start_transpose(
                        h2T_sb[:, dt_, :], h2_d[:, dt_ * P:(dt_ + 1) * P])

            # ---------------- Phase 6: MLP GEMM1 + gelu ------------------
            with (tc.tile_pool(name="gm2w", bufs=1) as g2w,
                  tc.tile_pool(name="gm2p", bufs=4, space="PSUM") as g2p):
                w1_sb = g2w.tile([P, DT, FT, P], bf16, name="w1_sb")  # 8 MB
                nc.sync.dma_start(w1_sb[:], w1_d[:])
                b1_sb = g2w.tile([P, FT], f32, name="b1_sb")
                nc.sync.dma_start(b1_sb[:], b1_d[:])
                for m in range(FT):
                    for n2 in range(2):
                        psB = g2p.tile([P, 512], f32, tag="psB", name="psB")
                        for k in range(DT):
                            nc.tensor.matmul(
                                psB[:], w1_sb[:, k, m, :],
                                h2T_sb[:, k, n2 * 512:(n2 + 1) * 512],
                                start=(k == 0), stop=(k == DT - 1))
                        nc.scalar.activation(
                            mlp1T_sb[:, m, n2 * 512:(n2 + 1) * 512], psB[:],
                            AF.Gelu_apprx_tanh, bias=b1_sb[:, m:m + 1])

            # ---------------- Phase 7: MLP GEMM2 + final residual --------
            with (tc.tile_pool(name="gm3w", bufs=1) as g3w,
                  tc.tile_pool(name="gm3t", bufs=2) as g3t,
                  tc.tile_pool(name="gm3p", bufs=4, space="PSUM") as g3p):
                w2_sb = g3w.tile([P, FT, DT, P], bf16, name="w2_sb")  # 8 MB
                nc.sync.dma_start(w2_sb[:], w2_d[:])
                b2_sb = g3w.tile([P, DT], f32, name="b2_sb")
                nc.sync.dma_start(b2_sb[:], b2_d[:])
                o2v = o2_d.rearrange("(dt p) t -> p dt t", p=P)
                for m in range(DT):
                    for n2 in range(2):
                        psC = g3p.tile([P, 512], f32, tag="psC", name="psC")
                        for k in range(FT):
                            nc.tensor.matmul(
                                psC[:], w2_sb[:, k, m, :],
                                mlp1T_sb[:, k, n2 * 512:(n2 + 1) * 512],
                                start=(k == 0), stop=(k == FT - 1))
                        o2s = g3t.tile([P, 512], bf16, tag="o2s", name="o2s")
                        nc.scalar.activation(o2s[:], psC[:], AF.Identity,
                                             bias=b2_sb[:, m:m + 1])
                        nc.sync.dma_start(
                            o2v[:, m, n2 * 512:(n2 + 1) * 512], o2s[:])
                for tt in range(DT):
                    otok = g3t.tile([P, D], bf16, tag="otok", name="otok")
                    nc.sync.dma_start_transpose(
                        otok[:], o2_d[:, tt * P:(tt + 1) * P])
                    yt = g3t.tile([P, D], f32, tag="yt", name="yt")
                    nc.vector.tensor_add(yt[:], x2_sb[:, tt, :], otok[:])
                    nc.sync.dma_start(y_out[tt * P:(tt + 1) * P, :], yt[:])
            pers_cm.__exit__(None, None, None)

    nc.compile()
    return nc


def _get_program():
    if "nc" not in _prog_cache:
        _prog_cache["nc"] = _build_program()
    return _prog_cache["nc"]


def _pack_km(w, kt, mt):
    """[kt*128, mt*128] -> [128, kt, mt, 128]"""
    return np.ascontiguousarray(
        w.reshape(kt, P, mt, P).transpose(1, 0, 2, 3))


def _gm(v):
    """[n*128] channel vector -> gate-major [128, n]"""
    n = v.shape[0] // P
    return np.ascontiguousarray(v.reshape(n, P).T)


def kernel(x, hx, ln1_g, ln1_b, W_ff1, b_ff1, W_ff2, b_ff2, W_ta, b_ta,
           W_tb, b_tb, lif_beta, lif_vth, ln2_g, ln2_b, W1, b1, W2, b2):
    global last_exec_time_ns
    x = np.asarray(x, np.float32)
    hx = np.asarray(hx, np.float32)

    f = lambda a: np.asarray(a, np.float32)
    W_t = TS * f(W_ta) + f(W_tb)
    b_t = TS * f(b_ta) + f(b_tb)
    Wall = np.concatenate([f(W_ff1), f(W_ff2), W_t], axis=1)  # [2560, 4608]
    bf = ml_dtypes.bfloat16
    wx_p = _pack_km(Wall[:D], DT, MT).astype(bf)
    wh_p = _pack_km(Wall[D:], KT, MT).astype(bf)
    bias_gm = _gm(np.concatenate([f(b_ff1), f(b_ff2), b_t]))
    w1_p = _pack_km(f(W1), DT, FT).astype(bf)
    w2_p = _pack_km(f(W2), FT, DT).astype(bf)

    common = dict(
        wh=wh_p, wx=wx_p, bias=bias_gm,
        ident=np.eye(P).astype(bf),
        beta=_gm(f(lif_beta)), vth=_gm(f(lif_vth)),
        ln1g=f(ln1_g), ln1b=f(ln1_b), ln2g=f(ln2_g), ln2b=f(ln2_b),
        w1=w1_p, b1=_gm(f(b1)), w2=w2_p, b2=_gm(f(b2)),
    )
    in_maps = []
    for b in range(B):
        m = dict(common)
        m["x"] = np.ascontiguousarray(x[b])
        m["hx"] = np.ascontiguousarray(hx[b].reshape(KT, P).T)
        in_maps.append(m)

    nc = _get_program()
    res = run_bass_kernel_spmd(nc, in_maps, core_ids=list(range(B)),
                               trace=TRACE)
    last_exec_time_ns = res.exec_time_ns

    x_out = np.stack([res.results[b]["y_out"] for b in range(B)])
    h_final = np.stack(
        [res.results[b]["h_out"].T.reshape(U) for b in range(B)])
    return x_out, h_final


# revision 17
# speedup vs baseline: 1.0125x; 1.0125x over previous
"""Trainium2 Bass kernel for nn_AudioLiquidEmberBlock.

Computation (per batch element, B=8 sharded across 8 NeuronCores):
  normed = LN1(x)                                      # [T=1024, D=1024]
  scan over T with CfC cell (tanh/tanh/sigmoid gates, U=1536)
  fused with LIF spiking gate on the first D hidden channels
  x = x + gated; x = x + MLP(LN2(x))                   # MLP d_ff=4096
  returns (x, h_final)

Strategy:
  - Batch-parallel over 8 cores (batch 8 -> 1 sequence per core).
  - The x-contribution of the CfC gates (x_t @ W_x + b) is hoisted out of
    the scan into one big GEMM; only the h-recurrence (h @ W_h) runs in the
    sequential loop. Since TS == 1.0, the ta/tb gates fold into a single
    combined gate matrix, so the scan does 3 gate matmuls, not 4.
  - Scan uses gate-major layout: weights stationary on the PE
    ([128,128] bf16 tiles, FWL), hidden state streams as rhs columns.
  - All matmuls bf16 with fp32 PSUM accumulation; LN/elementwise fp32.
"""
import os
import sys

sys.path.insert(0, '/opt/trn_rl_repo')

import numpy as np
import ml_dtypes

import concourse.bass as bass
import concourse.tile as tile
from concourse import bacc, mybir
from concourse.bass import ds
from concourse.bass_utils import run_bass_kernel_spmd

P = 128
D = 1024          # n_embd
U = 1536          # CfC units
DFF = 4096        # MLP hidden
T = 1024          # sequence length
B = 8             # batch == number of cores
KT = U // P       # 12 k-tiles for the recurrence
MT = 3 * (U // P)  # 36 m-tiles (3 gates after ta/tb fold)
DT = D // P       # 8
FT = DFF // P     # 32
UNROLL = 32
HALF = KT // 2
NCH = 256         # T-chunk for the xw precompute GEMM
EPS = 1e-5
SLOPE = 5.0
TS = 1.0

f32 = mybir.dt.float32
bf16 = mybir.dt.bfloat16
AF = mybir.ActivationFunctionType
ET = mybir.EngineType

TRACE = bool(int(os.environ.get("KERNEL_TRACE", "0")))
last_exec_time_ns = None

_prog_cache = {}


def _layer_norm(nc, pool, x_sb, g_bc, b_bc, out_sb, tag):
    """LN over the free dim (size D) of x_sb [P, D] fp32 -> out_sb."""
    mu = pool.tile([P, 1], f32, tag=f"{tag}_mu", name=f"{tag}_mu")
    nc.vector.reduce_sum(mu[:], x_sb[:], axis=mybir.AxisListType.X)
    nc.vector.tensor_scalar_mul(mu[:], mu[:], 1.0 / D)
    xc = pool.tile([P, D], f32, tag=f"{tag}_xc", name=f"{tag}_xc")
    nc.vector.tensor_scalar_sub(xc[:], x_sb[:], mu[:])
    sq = pool.tile([P, D], f32, tag=f"{tag}_sq", name=f"{tag}_sq")
    var = pool.tile([P, 1], f32, tag=f"{tag}_var", name=f"{tag}_var")
    nc.scalar.activation(sq[:], xc[:], AF.Square, accum_out=var[:])
    rstd = pool.tile([P, 1], f32, tag=f"{tag}_rstd", name=f"{tag}_rstd")
    nc.vector.tensor_scalar(
        out=rstd[:], in0=var[:], scalar1=1.0 / D, scalar2=EPS,
        op0=mybir.AluOpType.mult, op1=mybir.AluOpType.add)
    nc.scalar.activation(rstd[:], rstd[:], AF.Sqrt)
    nc.vector.reciprocal(rstd[:], rstd[:])
    nc.vector.tensor_scalar_mul(xc[:], xc[:], rstd[:])
    nc.vector.tensor_mul(xc[:], xc[:], g_bc)
    nc.vector.tensor_add(out_sb[:], xc[:], b_bc)


def _build_program():
    nc = bacc.Bacc("TRN2", target_bir_lowering=False, debug=False,
                   enable_asserts=False, num_devices=B)

    x_d = nc.dram_tensor("x", [T, D], f32, kind="ExternalInput")
    hx_d = nc.dram_tensor("hx", [P, KT], f32, kind="ExternalInput")
    id_d = nc.dram_tensor("ident", [P, P], bf16, kind="ExternalInput")
    wh_d = nc.dram_tensor("wh", [P, KT, MT, P], bf16, kind="ExternalInput")
    wx_d = nc.dram_tensor("wx", [P, DT, MT, P], bf16, kind="ExternalInput")
    bias_d = nc.dram_tensor("bias", [P, MT], f32, kind="ExternalInput")
    beta_d = nc.dram_tensor("beta", [P, DT], f32, kind="ExternalInput")
    vth_d = nc.dram_tensor("vth", [P, DT], f32, kind="ExternalInput")
    ln1g_d = nc.dram_tensor("ln1g", [D], f32, kind="ExternalInput")
    ln1b_d = nc.dram_tensor("ln1b", [D], f32, kind="ExternalInput")
    ln2g_d = nc.dram_tensor("ln2g", [D], f32, kind="ExternalInput")
    ln2b_d = nc.dram_tensor("ln2b", [D], f32, kind="ExternalInput")
    w1_d = nc.dram_tensor("w1", [P, DT, FT, P], bf16, kind="ExternalInput")
    b1_d = nc.dram_tensor("b1", [P, FT], f32, kind="ExternalInput")
    w2_d = nc.dram_tensor("w2", [P, FT, DT, P], bf16, kind="ExternalInput")
    b2_d = nc.dram_tensor("b2", [P, DT], f32, kind="ExternalInput")

    y_out = nc.dram_tensor("y_out", [T, D], f32, kind="ExternalOutput")
    h_out = nc.dram_tensor("h_out", [P, KT], f32, kind="ExternalOutput")

    # DRAM scratch
    nrm_d = nc.dram_tensor("nrm_scratch", [T, D], bf16)
    xw_d = nc.dram_tensor("xw_scratch", [P, T, MT], bf16)
    gd_d = nc.dram_tensor("gd_scratch", [D, T], bf16)
    h2_d = nc.dram_tensor("h2_scratch", [T, D], bf16)
    o2_d = nc.dram_tensor("o2_scratch", [D, T], bf16)

    with tile.TileContext(nc) as tc:
        if True:
            # ---------------- Phase 1: LN1, write normed (bf16) ----------
            with (tc.tile_pool(name="ln1", bufs=2) as lp,
                  tc.tile_pool(name="lnc", bufs=1) as lc):
                ln1g_sb = lc.tile([P, D], f32, name="ln1g_sb")
                nc.sync.dma_start(ln1g_sb[:], ln1g_d[None, :].to_broadcast([P, D]))
                ln1b_sb = lc.tile([P, D], f32, name="ln1b_sb")
                nc.sync.dma_start(ln1b_sb[:], ln1b_d[None, :].to_broadcast([P, D]))
                g_bc = ln1g_sb[:]
                b_bc = ln1b_sb[:]
                for tt in range(DT):
                    xt = lp.tile([P, D], f32, tag="xt", name="xt")
                    nc.sync.dma_start(xt[:], x_d[tt * P:(tt + 1) * P, :])
                    nrm = lp.tile([P, D], f32, tag="nrm", name="nrm")
                    _layer_norm(nc, lp, xt, g_bc, b_bc, nrm, "ln1")
                    nrmb = lp.tile([P, D], bf16, tag="nrmb", name="nrmb")
                    nc.vector.tensor_copy(nrmb[:], nrm[:])
                    nc.sync.dma_start(nrm_d[tt * P:(tt + 1) * P, :], nrmb[:])

            # ---------------- Phase 2+3: xw = normed @ Wx + b ------------
            with (tc.tile_pool(name="gm1w", bufs=1) as gw,
                  tc.tile_pool(name="gm1s", bufs=2) as gs,
                  tc.tile_pool(name="gm1p", bufs=4, space="PSUM") as gp):
                nrmT_sb = gw.tile([P, DT, T], bf16, name="nrmT_sb")  # 2 MB
                for dt_ in range(DT):
                    nc.sync.dma_start_transpose(
                        nrmT_sb[:, dt_, :], nrm_d[:, dt_ * P:(dt_ + 1) * P])
                wx_sb = gw.tile([P, DT, MT, P], bf16, name="wx_sb")  # 9.4 MB
                nc.sync.dma_start(wx_sb[:], wx_d[:])
                bias_sb = gw.tile([P, MT], f32, name="bias_sb")
                nc.sync.dma_start(bias_sb[:], bias_d[:])
                for n in range(T // NCH):
                    stg = gs.tile([P, NCH, MT], bf16, tag="stg", name="stg")
                    for m in range(MT):
                        psA = gp.tile([P, NCH], f32, tag="psA", name="psA")
                        for k in range(DT):
                            nc.tensor.matmul(
                                psA[:], wx_sb[:, k, m, :],
                                nrmT_sb[:, k, n * NCH:(n + 1) * NCH],
                                start=(k == 0), stop=(k == DT - 1))
                        nc.scalar.activation(stg[:, :, m], psA[:], AF.Identity,
                                             bias=bias_sb[:, m:m + 1])
                    nc.sync.dma_start(xw_d[:, n * NCH:(n + 1) * NCH, :], stg[:])

            # ---------------- Phase 4: the scan --------------------------
            with (tc.tile_pool(name="scw", bufs=1) as sw,
                  tc.tile_pool(name="scs", bufs=1) as st,
                  tc.tile_pool(name="scx", bufs=3) as sx,
                  tc.tile_pool(name="scwk", bufs=3) as wk,
                  tc.tile_pool(name="scps", bufs=2, space="PSUM") as pp):
                wh_sb = sw.tile([P, KT, MT, P], bf16, name="wh_sb")  # 14.2 MB
                nc.sync.dma_start(wh_sb[:], wh_d[:])
                id_sb = sw.tile([P, P], bf16, name="id_sb")
                nc.sync.dma_start(id_sb[:], id_d[:])
                beta_sb = st.tile([P, DT], f32, name="beta_sb")
                nc.sync.dma_start(beta_sb[:], beta_d[:])
                nc.scalar.activation(beta_sb[:], beta_sb[:], AF.Sigmoid)
                vth_sb = st.tile([P, DT], f32, name="vth_sb")
                nc.sync.dma_start(vth_sb[:], vth_d[:])
                gatedT = st.tile([P, DT, T], bf16, name="gatedT")   # 2 MB
                mem = st.tile([P, DT], f32, name="mem")
                nc.any.memset(mem[:], 0.0)
                hT = []
                for i in range(4):
                    ha = st.tile([P, HALF], bf16, name=f"h{i}a")
                    hb = st.tile([P, HALF], bf16, name=f"h{i}b")
                    hT.append((ha, hb))
                hf32 = st.tile([P, KT], f32, name="hf32")
                hx_sb = st.tile([P, KT], f32, name="hx_sb")
                nc.sync.dma_start(hx_sb[:], hx_d[:])
                nc.vector.tensor_copy(hT[0][0][:], hx_sb[:, 0:HALF])
                nc.vector.tensor_copy(hT[0][1][:], hx_sb[:, HALF:KT])
                nc.vector.tensor_copy(hf32[:], hx_sb[:])

                with tc.For_i(0, T, UNROLL, hint_engines=(ET.PE,),
                              staggered_reset=True) as t0:
                    xwc = sx.tile([P, UNROLL, MT], bf16, name="xwc")
                    nc.sync.dma_start(xwc[:], xw_d[:, ds(t0, UNROLL), :])

                    def init_psums(u):
                        pst = pp.tile([P, KT], f32, tag="pst", name="pst",
                                      padded_shape=[P, 512])
                        ps2 = pp.tile([P, KT], f32, tag="ps2", name="ps2",
                                      padded_shape=[P, 512])
                        ps1a = pp.tile([P, HALF], f32, tag="ps1a", name="ps1a",
                                       padded_shape=[P, 512])
                        ps1b = pp.tile([P, HALF], f32, tag="ps1b", name="ps1b",
                                       padded_shape=[P, 512])
                        nc.tensor.matmul(pst[:], id_sb[:],
                                         xwc[:, u, 2 * KT:3 * KT],
                                         start=True, stop=False)
                        nc.tensor.matmul(ps2[:], id_sb[:],
                                         xwc[:, u, KT:2 * KT],
                                         start=True, stop=False)
                        nc.tensor.matmul(ps1a[:], id_sb[:], xwc[:, u, 0:HALF],
                                         start=True, stop=False)
                        nc.tensor.matmul(ps1b[:], id_sb[:], xwc[:, u, HALF:KT],
                                         start=True, stop=False)
                        return pst, ps2, ps1a, ps1b

                    psums = init_psums(0)
                    for u in range(UNROLL):
                        curA, curB = hT[u % 4]
                        nxtA, nxtB = hT[(u + 1) % 4]

                        def rhs_col(k):
                            return curA[:, k:k + 1] if k < HALF else \
                                curB[:, k - HALF:k - HALF + 1]

                        pst, ps2, ps1a, ps1b = psums
                        # tc gate: K-OUTER (consumes cur column-by-column)
                        for k in range(KT):
                            for mi in range(KT):
                                nc.tensor.matmul(
                                    pst[:, mi:mi + 1],
                                    wh_sb[:, k, 2 * KT + mi, :],
                                    rhs_col(k), start=False,
                                    stop=(k == KT - 1), skip_group_check=True)
                        sg = wk.tile([P, KT], f32, tag="sg", name="sg")
                        nc.scalar.activation(sg[:], pst[:], AF.Sigmoid)
                        oms = wk.tile([P, KT], f32, tag="oms", name="oms")
                        nc.vector.tensor_scalar(
                            out=oms[:], in0=sg[:], scalar1=-1.0, scalar2=1.0,
                            op0=mybir.AluOpType.mult, op1=mybir.AluOpType.add)
                        if u + 1 < UNROLL:
                            psums = init_psums(u + 1)
                        # ff2 gate: m-outer
                        for mi in range(KT):
                            for k in range(KT):
                                nc.tensor.matmul(
                                    ps2[:, mi:mi + 1], wh_sb[:, k, KT + mi, :],
                                    rhs_col(k), start=False,
                                    stop=(k == KT - 1))
                        ff2 = wk.tile([P, KT], f32, tag="ff2", name="ff2")
                        nc.scalar.activation(ff2[:], ps2[:], AF.Tanh)
                        cc = wk.tile([P, KT], f32, tag="cc", name="cc")
                        nc.vector.tensor_mul(cc[:], sg[:], ff2[:])
                        # ff1 gate: m-outer, tail in column halves
                        ff1 = wk.tile([P, KT], f32, tag="ff1", name="ff1")
                        t1 = wk.tile([P, KT], f32, tag="t1", name="t1")
                        for half, (psh, nxth) in enumerate(((ps1a, nxtA),
                                                            (ps1b, nxtB))):
                            sl = slice(half * HALF, (half + 1) * HALF)
                            for mj in range(HALF):
                                mi = half * HALF + mj
                                for k in range(KT):
                                    nc.tensor.matmul(
                                        psh[:, mj:mj + 1], wh_sb[:, k, mi, :],
                                        rhs_col(k), start=False,
                                        stop=(k == KT - 1))
                            nc.scalar.activation(ff1[:, sl], psh[:], AF.Tanh)
                            nc.vector.tensor_mul(t1[:, sl], ff1[:, sl],
                                                 oms[:, sl])
                            nc.vector.tensor_add(nxth[:], t1[:, sl], cc[:, sl])
                        # off path: fp32 h for LIF + h_final
                        nc.vector.tensor_add(hf32[:], t1[:], cc[:])
                        # LIF membrane update on the first DT tiles
                        m1 = wk.tile([P, DT], f32, tag="m1", name="m1")
                        nc.vector.tensor_mul(m1[:], mem[:], beta_sb[:])
                        nc.vector.tensor_add(m1[:], m1[:], hf32[:, 0:DT])
                        spin = wk.tile([P, DT], f32, tag="spin", name="spin")
                        nc.vector.tensor_sub(spin[:], m1[:], vth_sb[:])
                        spk = wk.tile([P, DT], f32, tag="spk", name="spk")
                        nc.scalar.activation(spk[:], spin[:], AF.Sigmoid,
                                             scale=SLOPE)
                        sv = wk.tile([P, DT], f32, tag="sv", name="sv")
                        nc.vector.tensor_mul(sv[:], spk[:], vth_sb[:])
                        nc.vector.tensor_sub(mem[:], m1[:], sv[:])
                        nc.vector.tensor_mul(gatedT[:, :, ds(t0 + u, 1)],
                                             hf32[:, 0:DT, None],
                                             spk[:, :, None])
                nc.sync.dma_start(h_out[:], hf32[:])
                nc.sync.dma_start(
                    gd_d.rearrange("(dt p) t -> p dt t", p=P), gatedT[:])

            # ---------------- Phase 5: residual + LN2 --------------------
            pers_cm = tc.tile_pool(name="persist", bufs=1)
            pers = pers_cm.__enter__()
            x2_sb = pers.tile([P, DT, D], f32, name="x2_sb")       # 4 MB
            h2T_sb = pers.tile([P, DT, T], bf16, name="h2T_sb")    # 2 MB
            mlp1T_sb = pers.tile([P, FT, T], bf16, name="mlp1T_sb")  # 8 MB
            with (tc.tile_pool(name="ln2", bufs=2) as lp2,
                  tc.tile_pool(name="lnc2", bufs=1) as lc2):
                ln2g_sb = lc2.tile([P, D], f32, name="ln2g_sb")
                nc.sync.dma_start(ln2g_sb[:], ln2g_d[None, :].to_broadcast([P, D]))
                ln2b_sb = lc2.tile([P, D], f32, name="ln2b_sb")
                nc.sync.dma_start(ln2b_sb[:], ln2b_d[None, :].to_broadcast([P, D]))
                g2_bc = ln2g_sb[:]
                b2_bc = ln2b_sb[:]
                for tt in range(DT):
                    gtok = lp2.tile([P, D], bf16, tag="gtok", name="gtok")
                    nc.sync.dma_start_transpose(
                        gtok[:], gd_d[:, tt * P:(tt + 1) * P])
                    xt2 = lp2.tile([P, D], f32, tag="xt2", name="xt2")
                    nc.sync.dma_start(xt2[:], x_d[tt * P:(tt + 1) * P, :])
                    nc.vector.tensor_add(x2_sb[:, tt, :], xt2[:], gtok[:])
                    h2t = lp2.tile([P, D], f32, tag="h2t", name="h2t")
                    _layer_norm(nc, lp2, x2_sb[:, tt, :], g2_bc, b2_bc, h2t,
                                "ln2")
                    h2b = lp2.tile([P, D], bf16, tag="h2b", name="h2b")
                    nc.vector.tensor_copy(h2b[:], h2t[:])
                    nc.sync.dma_start(h2_d[tt * P:(tt + 1) * P, :], h2b[:])
                for dt_ in range(DT):
                    nc.sync.dma_start_transpose(
                        h2T_sb[:, dt_, :], h2_d[:, dt_ * P:(dt_ + 1) * P])

            # ---------------- Phase 6: MLP GEMM1 + gelu ------------------
            with (tc.tile_pool(name="gm2w", bufs=1) as g2w,
                  tc.tile_pool(name="gm2p", bufs=4, space="PSUM") as g2p):
                w1_sb = g2w.tile([P, DT, FT, P], bf16, name="w1_sb")  # 8 MB
                nc.sync.dma_start(w1_sb[:], w1_d[:])
                b1_sb = g2w.tile([P, FT], f32, name="b1_sb")
                nc.sync.dma_start(b1_sb[:], b1_d[:])
                for m in range(FT):
                    for n2 in range(2):
                        psB = g2p.tile([P, 512], f32, tag="psB", name="psB")
                        for k in range(DT):
                            nc.tensor.matmul(
                                psB[:], w1_sb[:, k, m, :],
                                h2T_sb[:, k, n2 * 512:(n2 + 1) * 512],
                                start=(k == 0), stop=(k == DT - 1))
                        nc.scalar.activation(
                            mlp1T_sb[:, m, n2 * 512:(n2 + 1) * 512], psB[:],
                            AF.Gelu_apprx_tanh, bias=b1_sb[:, m:m + 1])

            # ---------------- Phase 7: MLP GEMM2 + final residual --------
            with (tc.tile_pool(name="gm3w", bufs=1) as g3w,
                  tc.tile_pool(name="gm3t", bufs=2) as g3t,
                  tc.tile_pool(name="gm3p", bufs=4, space="PSUM") as g3p):
                w2_sb = g3w.tile([P, FT, DT, P], bf16, name="w2_sb")  # 8 MB
                nc.sync.dma_start(w2_sb[:], w2_d[:])
                b2_sb = g3w.tile([P, DT], f32, name="b2_sb")
                nc.sync.dma_start(b2_sb[:], b2_d[:])
                o2v = o2_d.rearrange("(dt p) t -> p dt t", p=P)
                for m in range(DT):
                    for n2 in range(2):
                        psC = g3p.tile([P, 512], f32, tag="psC", name="psC")
                        for k in range(FT):
                            nc.tensor.matmul(
                                psC[:], w2_sb[:, k, m, :],
                                mlp1T_sb[:, k, n2 * 512:(n2 + 1) * 512],
                                start=(k == 0), stop=(k == FT - 1))
                        o2s = g3t.tile([P, 512], bf16, tag="o2s", name="o2s")
                        nc.scalar.activation(o2s[:], psC[:], AF.Identity,
                                             bias=b2_sb[:, m:m + 1])
                        nc.sync.dma_start(
                            o2v[:, m, n2 * 512:(n2 + 1) * 512], o2s[:])
                for tt in range(DT):
                    otok = g3t.tile([P, D], bf16, tag="otok", name="otok")
                    nc.sync.dma_start_transpose(
                        otok[:], o2_d[:, tt * P:(tt + 1) * P])
                    yt = g3t.tile([P, D], f32, tag="yt", name="yt")
                    nc.vector.tensor_add(yt[:], x2_sb[:, tt, :], otok[:])
                    nc.sync.dma_start(y_out[tt * P:(tt + 1) * P, :], yt[:])
            pers_cm.__exit__(None, None, None)

    nc.compile()
    return nc


def _get_program():
    if "nc" not in _prog_cache:
        _prog_cache["nc"] = _build_program()
    return _prog_cache["nc"]


def _pack_km(w, kt, mt):
    """[kt*128, mt*128] -> [128, kt, mt, 128]"""
    return np.ascontiguousarray(
        w.reshape(kt, P, mt, P).transpose(1, 0, 2, 3))


def _gm(v):
    """[n*128] channel vector -> gate-major [128, n]"""
    n = v.shape[0] // P
    return np.ascontiguousarray(v.reshape(n, P).T)


def kernel(x, hx, ln1_g, ln1_b, W_ff1, b_ff1, W_ff2, b_ff2, W_ta, b_ta,
           W_tb, b_tb, lif_beta, lif_vth, ln2_g, ln2_b, W1, b1, W2, b2):
    global last_exec_time_ns
    x = np.asarray(x, np.float32)
    hx = np.asarray(hx, np.float32)

    f = lambda a: np.asarray(a, np.float32)
    W_t = TS * f(W_ta) + f(W_tb)
    b_t = TS * f(b_ta) + f(b_tb)
    Wall = np.concatenate([f(W_ff1), f(W_ff2), W_t], axis=1)  # [2560, 4608]
    bf = ml_dtypes.bfloat16
    wx_p = _pack_km(Wall[:D], DT, MT).astype(bf)
    wh_p = _pack_km(Wall[D:], KT, MT).astype(bf)
    bias_gm = _gm(np.concatenate([f(b_ff1), f(b_ff2), b_t]))
    w1_p = _pack_km(f(W1), DT, FT).astype(bf)
    w2_p = _pack_km(f(W2), FT, DT).astype(bf)

    common = dict(
        wh=wh_p, wx=wx_p, bias=bias_gm,
        ident=np.eye(P).astype(bf),
        beta=_gm(f(lif_beta)), vth=_gm(f(lif_vth)),
        ln1g=f(ln1_g), ln1b=f(ln1_b), ln2g=f(ln2_g), ln2b=f(ln2_b),
        w1=w1_p, b1=_gm(f(b1)), w2=w2_p, b2=_gm(f(b2)),
    )
    in_maps = []
    for b in range(B):
        m = dict(common)
        m["x"] = np.ascontiguousarray(x[b])
        m["hx"] = np.ascontiguousarray(hx[b].reshape(KT, P).T)
        in_maps.append(m)

    nc = _get_program()
    res = run_bass_kernel_spmd(nc, in_maps, core_ids=list(range(B)),
                               trace=TRACE)
    last_exec_time_ns = res.exec_time_ns

    x_out = np.stack([res.results[b]["y_out"] for b in range(B)])
    h_final = np.stack(
        [res.results[b]["h_out"].T.reshape(U) for b in range(B)])
    return x_out, h_final


# revision 18
# speedup vs baseline: 1.0181x; 1.0056x over previous
"""Trainium2 Bass kernel for nn_AudioLiquidEmberBlock.

Computation (per batch element, B=8 sharded across 8 NeuronCores):
  normed = LN1(x)                                      # [T=1024, D=1024]
  scan over T with CfC cell (tanh/tanh/sigmoid gates, U=1536)
  fused with LIF spiking gate on the first D hidden channels
  x = x + gated; x = x + MLP(LN2(x))                   # MLP d_ff=4096
  returns (x, h_final)

Strategy:
  - Batch-parallel over 8 cores (batch 8 -> 1 sequence per core).
  - The x-contribution of the CfC gates (x_t @ W_x + b) is hoisted out of
    the scan into one big GEMM; only the h-recurrence (h @ W_h) runs in the
    sequential loop. Since TS == 1.0, the ta/tb gates fold into a single
    combined gate matrix, so the scan does 3 gate matmuls, not 4.
  - Scan uses gate-major layout: weights stationary on the PE
    ([128,128] bf16 tiles, FWL), hidden state streams as rhs columns.
  - All matmuls bf16 with fp32 PSUM accumulation; LN/elementwise fp32.
"""
import os
import sys

sys.path.insert(0, '/opt/trn_rl_repo')

import numpy as np
import ml_dtypes

import concourse.bass as bass
import concourse.tile as tile
from concourse import bacc, mybir
from concourse.bass import ds
from concourse.bass_utils import run_bass_kernel_spmd

P = 128
D = 1024          # n_embd
U = 1536          # CfC units
DFF = 4096        # MLP hidden
T = 1024          # sequence length
B = 8             # batch == number of cores
KT = U // P       # 12 k-tiles for the recurrence
MT = 3 * (U // P)  # 36 m-tiles (3 gates after ta/tb fold)
DT = D // P       # 8
FT = DFF // P     # 32
UNROLL = 64
HALF = KT // 2
NCH = 256         # T-chunk for the xw precompute GEMM
EPS = 1e-5
SLOPE = 5.0
TS = 1.0

f32 = mybir.dt.float32
bf16 = mybir.dt.bfloat16
AF = mybir.ActivationFunctionType
ET = mybir.EngineType

TRACE = bool(int(os.environ.get("KERNEL_TRACE", "0")))
last_exec_time_ns = None

_prog_cache = {}


def _layer_norm(nc, pool, x_sb, g_bc, b_bc, out_sb, tag):
    """LN over the free dim (size D) of x_sb [P, D] fp32 -> out_sb."""
    mu = pool.tile([P, 1], f32, tag=f"{tag}_mu", name=f"{tag}_mu")
    nc.vector.reduce_sum(mu[:], x_sb[:], axis=mybir.AxisListType.X)
    nc.vector.tensor_scalar_mul(mu[:], mu[:], 1.0 / D)
    xc = pool.tile([P, D], f32, tag=f"{tag}_xc", name=f"{tag}_xc")
    nc.vector.tensor_scalar_sub(xc[:], x_sb[:], mu[:])
    sq = pool.tile([P, D], f32, tag=f"{tag}_sq", name=f"{tag}_sq")
    var = pool.tile([P, 1], f32, tag=f"{tag}_var", name=f"{tag}_var")
    nc.scalar.activation(sq[:], xc[:], AF.Square, accum_out=var[:])
    rstd = pool.tile([P, 1], f32, tag=f"{tag}_rstd", name=f"{tag}_rstd")
    nc.vector.tensor_scalar(
        out=rstd[:], in0=var[:], scalar1=1.0 / D, scalar2=EPS,
        op0=mybir.AluOpType.mult, op1=mybir.AluOpType.add)
    nc.scalar.activation(rstd[:], rstd[:], AF.Sqrt)
    nc.vector.reciprocal(rstd[:], rstd[:])
    nc.vector.tensor_scalar_mul(xc[:], xc[:], rstd[:])
    nc.vector.tensor_mul(xc[:], xc[:], g_bc)
    nc.vector.tensor_add(out_sb[:], xc[:], b_bc)


def _build_program():
    nc = bacc.Bacc("TRN2", target_bir_lowering=False, debug=False,
                   enable_asserts=False, num_devices=B)

    x_d = nc.dram_tensor("x", [T, D], f32, kind="ExternalInput")
    hx_d = nc.dram_tensor("hx", [P, KT], f32, kind="ExternalInput")
    id_d = nc.dram_tensor("ident", [P, P], bf16, kind="ExternalInput")
    wh_d = nc.dram_tensor("wh", [P, KT, MT, P], bf16, kind="ExternalInput")
    wx_d = nc.dram_tensor("wx", [P, DT, MT, P], bf16, kind="ExternalInput")
    bias_d = nc.dram_tensor("bias", [P, MT], f32, kind="ExternalInput")
    beta_d = nc.dram_tensor("beta", [P, DT], f32, kind="ExternalInput")
    vth_d = nc.dram_tensor("vth", [P, DT], f32, kind="ExternalInput")
    ln1g_d = nc.dram_tensor("ln1g", [D], f32, kind="ExternalInput")
    ln1b_d = nc.dram_tensor("ln1b", [D], f32, kind="ExternalInput")
    ln2g_d = nc.dram_tensor("ln2g", [D], f32, kind="ExternalInput")
    ln2b_d = nc.dram_tensor("ln2b", [D], f32, kind="ExternalInput")
    w1_d = nc.dram_tensor("w1", [P, DT, FT, P], bf16, kind="ExternalInput")
    b1_d = nc.dram_tensor("b1", [P, FT], f32, kind="ExternalInput")
    w2_d = nc.dram_tensor("w2", [P, FT, DT, P], bf16, kind="ExternalInput")
    b2_d = nc.dram_tensor("b2", [P, DT], f32, kind="ExternalInput")

    y_out = nc.dram_tensor("y_out", [T, D], f32, kind="ExternalOutput")
    h_out = nc.dram_tensor("h_out", [P, KT], f32, kind="ExternalOutput")

    # DRAM scratch
    nrm_d = nc.dram_tensor("nrm_scratch", [T, D], bf16)
    xw_d = nc.dram_tensor("xw_scratch", [P, T, MT], bf16)
    gd_d = nc.dram_tensor("gd_scratch", [D, T], bf16)
    h2_d = nc.dram_tensor("h2_scratch", [T, D], bf16)
    o2_d = nc.dram_tensor("o2_scratch", [D, T], bf16)

    with tile.TileContext(nc) as tc:
        if True:
            # ---------------- Phase 1: LN1, write normed (bf16) ----------
            with (tc.tile_pool(name="ln1", bufs=2) as lp,
                  tc.tile_pool(name="lnc", bufs=1) as lc):
                ln1g_sb = lc.tile([P, D], f32, name="ln1g_sb")
                nc.sync.dma_start(ln1g_sb[:], ln1g_d[None, :].to_broadcast([P, D]))
                ln1b_sb = lc.tile([P, D], f32, name="ln1b_sb")
                nc.sync.dma_start(ln1b_sb[:], ln1b_d[None, :].to_broadcast([P, D]))
                g_bc = ln1g_sb[:]
                b_bc = ln1b_sb[:]
                for tt in range(DT):
                    xt = lp.tile([P, D], f32, tag="xt", name="xt")
                    nc.sync.dma_start(xt[:], x_d[tt * P:(tt + 1) * P, :])
                    nrm = lp.tile([P, D], f32, tag="nrm", name="nrm")
                    _layer_norm(nc, lp, xt, g_bc, b_bc, nrm, "ln1")
                    nrmb = lp.tile([P, D], bf16, tag="nrmb", name="nrmb")
                    nc.vector.tensor_copy(nrmb[:], nrm[:])
                    nc.sync.dma_start(nrm_d[tt * P:(tt + 1) * P, :], nrmb[:])

            # ---------------- Phase 2+3: xw = normed @ Wx + b ------------
            with (tc.tile_pool(name="gm1w", bufs=1) as gw,
                  tc.tile_pool(name="gm1s", bufs=2) as gs,
                  tc.tile_pool(name="gm1p", bufs=4, space="PSUM") as gp):
                nrmT_sb = gw.tile([P, DT, T], bf16, name="nrmT_sb")  # 2 MB
                for dt_ in range(DT):
                    nc.sync.dma_start_transpose(
                        nrmT_sb[:, dt_, :], nrm_d[:, dt_ * P:(dt_ + 1) * P])
                wx_sb = gw.tile([P, DT, MT, P], bf16, name="wx_sb")  # 9.4 MB
                nc.sync.dma_start(wx_sb[:], wx_d[:])
                bias_sb = gw.tile([P, MT], f32, name="bias_sb")
                nc.sync.dma_start(bias_sb[:], bias_d[:])
                for n in range(T // NCH):
                    stg = gs.tile([P, NCH, MT], bf16, tag="stg", name="stg")
                    for m in range(MT):
                        psA = gp.tile([P, NCH], f32, tag="psA", name="psA")
                        for k in range(DT):
                            nc.tensor.matmul(
                                psA[:], wx_sb[:, k, m, :],
                                nrmT_sb[:, k, n * NCH:(n + 1) * NCH],
                                start=(k == 0), stop=(k == DT - 1))
                        nc.scalar.activation(stg[:, :, m], psA[:], AF.Identity,
                                             bias=bias_sb[:, m:m + 1])
                    nc.sync.dma_start(xw_d[:, n * NCH:(n + 1) * NCH, :], stg[:])

            # ---------------- Phase 4: the scan --------------------------
            with (tc.tile_pool(name="scw", bufs=1) as sw,
                  tc.tile_pool(name="scs", bufs=1) as st,
                  tc.tile_pool(name="scx", bufs=3) as sx,
                  tc.tile_pool(name="scwk", bufs=3) as wk,
                  tc.tile_pool(name="scps", bufs=2, space="PSUM") as pp):
                wh_sb = sw.tile([P, KT, MT, P], bf16, name="wh_sb")  # 14.2 MB
                nc.sync.dma_start(wh_sb[:], wh_d[:])
                id_sb = sw.tile([P, P], bf16, name="id_sb")
                nc.sync.dma_start(id_sb[:], id_d[:])
                beta_sb = st.tile([P, DT], f32, name="beta_sb")
                nc.sync.dma_start(beta_sb[:], beta_d[:])
                nc.scalar.activation(beta_sb[:], beta_sb[:], AF.Sigmoid)
                vth_sb = st.tile([P, DT], f32, name="vth_sb")
                nc.sync.dma_start(vth_sb[:], vth_d[:])
                gatedT = st.tile([P, DT, T], bf16, name="gatedT")   # 2 MB
                mem = st.tile([P, DT], f32, name="mem")
                nc.any.memset(mem[:], 0.0)
                hT = []
                for i in range(4):
                    ha = st.tile([P, HALF], bf16, name=f"h{i}a")
                    hb = st.tile([P, HALF], bf16, name=f"h{i}b")
                    hT.append((ha, hb))
                hf32 = st.tile([P, KT], f32, name="hf32")
                hx_sb = st.tile([P, KT], f32, name="hx_sb")
                nc.sync.dma_start(hx_sb[:], hx_d[:])
                nc.vector.tensor_copy(hT[0][0][:], hx_sb[:, 0:HALF])
                nc.vector.tensor_copy(hT[0][1][:], hx_sb[:, HALF:KT])
                nc.vector.tensor_copy(hf32[:], hx_sb[:])

                with tc.For_i(0, T, UNROLL, hint_engines=(ET.PE,),
                              staggered_reset=True) as t0:
                    xwc = sx.tile([P, UNROLL, MT], bf16, name="xwc")
                    nc.sync.dma_start(xwc[:], xw_d[:, ds(t0, UNROLL), :])

                    def init_psums(u):
                        pst = pp.tile([P, KT], f32, tag="pst", name="pst",
                                      padded_shape=[P, 512])
                        ps2 = pp.tile([P, KT], f32, tag="ps2", name="ps2",
                                      padded_shape=[P, 512])
                        ps1a = pp.tile([P, HALF], f32, tag="ps1a", name="ps1a",
                                       padded_shape=[P, 512])
                        ps1b = pp.tile([P, HALF], f32, tag="ps1b", name="ps1b",
                                       padded_shape=[P, 512])
                        nc.tensor.matmul(pst[:], id_sb[:],
                                         xwc[:, u, 2 * KT:3 * KT],
                                         start=True, stop=False)
                        nc.tensor.matmul(ps2[:], id_sb[:],
                                         xwc[:, u, KT:2 * KT],
                                         start=True, stop=False)
                        nc.tensor.matmul(ps1a[:], id_sb[:], xwc[:, u, 0:HALF],
                                         start=True, stop=False)
                        nc.tensor.matmul(ps1b[:], id_sb[:], xwc[:, u, HALF:KT],
                                         start=True, stop=False)
                        return pst, ps2, ps1a, ps1b

                    psums = init_psums(0)
                    for u in range(UNROLL):
                        curA, curB = hT[u % 4]
                        nxtA, nxtB = hT[(u + 1) % 4]

                        def rhs_col(k):
                            return curA[:, k:k + 1] if k < HALF else \
                                curB[:, k - HALF:k - HALF + 1]

                        pst, ps2, ps1a, ps1b = psums
                        # tc gate: K-OUTER (consumes cur column-by-column)
                        for k in range(KT):
                            for mi in range(KT):
                                nc.tensor.matmul(
                                    pst[:, mi:mi + 1],
                                    wh_sb[:, k, 2 * KT + mi, :],
                                    rhs_col(k), start=False,
                                    stop=(k == KT - 1), skip_group_check=True)
                        sg = wk.tile([P, KT], f32, tag="sg", name="sg")
                        nc.scalar.activation(sg[:], pst[:], AF.Sigmoid)
                        oms = wk.tile([P, KT], f32, tag="oms", name="oms")
                        nc.vector.tensor_scalar(
                            out=oms[:], in0=sg[:], scalar1=-1.0, scalar2=1.0,
                            op0=mybir.AluOpType.mult, op1=mybir.AluOpType.add)
                        if u + 1 < UNROLL:
                            psums = init_psums(u + 1)
                        # ff2 gate: m-outer
                        for mi in range(KT):
                            for k in range(KT):
                                nc.tensor.matmul(
                                    ps2[:, mi:mi + 1], wh_sb[:, k, KT + mi, :],
                                    rhs_col(k), start=False,
                                    stop=(k == KT - 1))
                        ff2 = wk.tile([P, KT], f32, tag="ff2", name="ff2")
                        nc.scalar.activation(ff2[:], ps2[:], AF.Tanh)
                        cc = wk.tile([P, KT], f32, tag="cc", name="cc")
                        nc.vector.tensor_mul(cc[:], sg[:], ff2[:])
                        # ff1 gate: m-outer, tail in column halves
                        ff1 = wk.tile([P, KT], f32, tag="ff1", name="ff1")
                        t1 = wk.tile([P, KT], f32, tag="t1", name="t1")
                        for half, (psh, nxth) in enumerate(((ps1a, nxtA),
                                                            (ps1b, nxtB))):
                            sl = slice(half * HALF, (half + 1) * HALF)
                            for mj in range(HALF):
                                mi = half * HALF + mj
                                for k in range(KT):
                                    nc.tensor.matmul(
                                        psh[:, mj:mj + 1], wh_sb[:, k, mi, :],
                                        rhs_col(k), start=False,
                                        stop=(k == KT - 1))
                            nc.scalar.activation(ff1[:, sl], psh[:], AF.Tanh)
                            nc.vector.tensor_mul(t1[:, sl], ff1[:, sl],
                                                 oms[:, sl])
                            nc.vector.tensor_add(nxth[:], t1[:, sl], cc[:, sl])
                        # off path: fp32 h for LIF + h_final
                        nc.vector.tensor_add(hf32[:], t1[:], cc[:])
                        # LIF membrane update on the first DT tiles
                        m1 = wk.tile([P, DT], f32, tag="m1", name="m1")
                        nc.vector.tensor_mul(m1[:], mem[:], beta_sb[:])
                        nc.vector.tensor_add(m1[:], m1[:], hf32[:, 0:DT])
                        spin = wk.tile([P, DT], f32, tag="spin", name="spin")
                        nc.vector.tensor_sub(spin[:], m1[:], vth_sb[:])
                        spk = wk.tile([P, DT], f32, tag="spk", name="spk")
                        nc.scalar.activation(spk[:], spin[:], AF.Sigmoid,
                                             scale=SLOPE)
                        sv = wk.tile([P, DT], f32, tag="sv", name="sv")
                        nc.vector.tensor_mul(sv[:], spk[:], vth_sb[:])
                        nc.vector.tensor_sub(mem[:], m1[:], sv[:])
                        nc.vector.tensor_mul(gatedT[:, :, ds(t0 + u, 1)],
                                             hf32[:, 0:DT, None],
                                             spk[:, :, None])
                nc.sync.dma_start(h_out[:], hf32[:])
                nc.sync.dma_start(
                    gd_d.rearrange("(dt p) t -> p dt t", p=P), gatedT[:])

            # ---------------- Phase 5: residual + LN2 --------------------
            pers_cm = tc.tile_pool(name="persist", bufs=1)
            pers = pers_cm.__enter__()
            x2_sb = pers.tile([P, DT, D], f32, name="x2_sb")       # 4 MB
            h2T_sb = pers.tile([P, DT, T], bf16, name="h2T_sb")    # 2 MB
            mlp1T_sb = pers.tile([P, FT, T], bf16, name="mlp1T_sb")  # 8 MB
            with (tc.tile_pool(name="ln2", bufs=2) as lp2,
                  tc.tile_pool(name="lnc2", bufs=1) as lc2):
                ln2g_sb = lc2.tile([P, D], f32, name="ln2g_sb")
                nc.sync.dma_start(ln2g_sb[:], ln2g_d[None, :].to_broadcast([P, D]))
                ln2b_sb = lc2.tile([P, D], f32, name="ln2b_sb")
                nc.sync.dma_start(ln2b_sb[:], ln2b_d[None, :].to_broadcast([P, D]))
                g2_bc = ln2g_sb[:]
                b2_bc = ln2b_sb[:]
                for tt in range(DT):
                    gtok = lp2.tile([P, D], bf16, tag="gtok", name="gtok")
                    nc.sync.dma_start_transpose(
                        gtok[:], gd_d[:, tt * P:(tt + 1) * P])
                    xt2 = lp2.tile([P, D], f32, tag="xt2", name="xt2")
                    nc.sync.dma_start(xt2[:], x_d[tt * P:(tt + 1) * P, :])
                    nc.vector.tensor_add(x2_sb[:, tt, :], xt2[:], gtok[:])
                    h2t = lp2.tile([P, D], f32, tag="h2t", name="h2t")
                    _layer_norm(nc, lp2, x2_sb[:, tt, :], g2_bc, b2_bc, h2t,
                                "ln2")
                    h2b = lp2.tile([P, D], bf16, tag="h2b", name="h2b")
                    nc.vector.tensor_copy(h2b[:], h2t[:])
                    nc.sync.dma_start(h2_d[tt * P:(tt + 1) * P, :], h2b[:])
                for dt_ in range(DT):
                    nc.sync.dma_start_transpose(
                        h2T_sb[:, dt_, :], h2_d[:, dt_ * P:(dt_ + 1) * P])

            # ---------------- Phase 6: MLP GEMM1 + gelu ------------------
            with (tc.tile_pool(name="gm2w", bufs=1) as g2w,
                  tc.tile_pool(name="gm2p", bufs=4, space="PSUM") as g2p):
                w1_sb = g2w.tile([P, DT, FT, P], bf16, name="w1_sb")  # 8 MB
                nc.sync.dma_start(w1_sb[:], w1_d[:])
                b1_sb = g2w.tile([P, FT], f32, name="b1_sb")
                nc.sync.dma_start(b1_sb[:], b1_d[:])
                for m in range(FT):
                    for n2 in range(2):
                        psB = g2p.tile([P, 512], f32, tag="psB", name="psB")
                        for k in range(DT):
                            nc.tensor.matmul(
                                psB[:], w1_sb[:, k, m, :],
                                h2T_sb[:, k, n2 * 512:(n2 + 1) * 512],
                                start=(k == 0), stop=(k == DT - 1))
                        nc.scalar.activation(
                            mlp1T_sb[:, m, n2 * 512:(n2 + 1) * 512], psB[:],
                            AF.Gelu_apprx_tanh, bias=b1_sb[:, m:m + 1])

            # ---------------- Phase 7: MLP GEMM2 + final residual --------
            with (tc.tile_pool(name="gm3w", bufs=1) as g3w,
                  tc.tile_pool(name="gm3t", bufs=2) as g3t,
                  tc.tile_pool(name="gm3p", bufs=4, space="PSUM") as g3p):
                w2_sb = g3w.tile([P, FT, DT, P], bf16, name="w2_sb")  # 8 MB
                nc.sync.dma_start(w2_sb[:], w2_d[:])
                b2_sb = g3w.tile([P, DT], f32, name="b2_sb")
                nc.sync.dma_start(b2_sb[:], b2_d[:])
                o2v = o2_d.rearrange("(dt p) t -> p dt t", p=P)
                for m in range(DT):
                    for n2 in range(2):
                        psC = g3p.tile([P, 512], f32, tag="psC", name="psC")
                        for k in range(FT):
                            nc.tensor.matmul(
                                psC[:], w2_sb[:, k, m, :],
                                mlp1T_sb[:, k, n2 * 512:(n2 + 1) * 512],
                                start=(k == 0), stop=(k == FT - 1))
                        o2s = g3t.tile([P, 512], bf16, tag="o2s", name="o2s")
                        nc.scalar.activation(o2s[:], psC[:], AF.Identity,
                                             bias=b2_sb[:, m:m + 1])
                        nc.sync.dma_start(
                            o2v[:, m, n2 * 512:(n2 + 1) * 512], o2s[:])
                for tt in range(DT):
                    otok = g3t.tile([P, D], bf16, tag="otok", name="otok")
                    nc.sync.dma_start_transpose(
                        otok[:], o2_d[:, tt * P:(tt + 1) * P])
                    yt = g3t.tile([P, D], f32, tag="yt", name="yt")
                    nc.vector.tensor_add(yt[:], x2_sb[:, tt, :], otok[:])
                    nc.sync.dma_start(y_out[tt * P:(tt + 1) * P, :], yt[:])
            pers_cm.__exit__(None, None, None)

    nc.compile()
    return nc


def _get_program():
    if "nc" not in _prog_cache:
        _prog_cache["nc"] = _build_program()
    return _prog_cache["nc"]


def _pack_km(w, kt, mt):
    """[kt*128, mt*128] -> [128, kt, mt, 128]"""
    return np.ascontiguousarray(
        w.reshape(kt, P, mt, P).transpose(1, 0, 2, 3))


def _gm(v):
    """[n*128] channel vector -> gate-major [128, n]"""
    n = v.shape[0] // P
    return np.ascontiguousarray(v.reshape(n, P).T)


def kernel(x, hx, ln1_g, ln1_b, W_ff1, b_ff1, W_ff2, b_ff2, W_ta, b_ta,
           W_tb, b_tb, lif_beta, lif_vth, ln2_g, ln2_b, W1, b1, W2, b2):
    global last_exec_time_ns
    x = np.asarray(x, np.float32)
    hx = np.asarray(hx, np.float32)

    f = lambda a: np.asarray(a, np.float32)
    W_t = TS * f(W_ta) + f(W_tb)
    b_t = TS * f(b_ta) + f(b_tb)
    Wall = np.concatenate([f(W_ff1), f(W_ff2), W_t], axis=1)  # [2560, 4608]
    bf = ml_dtypes.bfloat16
    wx_p = _pack_km(Wall[:D], DT, MT).astype(bf)
    wh_p = _pack_km(Wall[D:], KT, MT).astype(bf)
    bias_gm = _gm(np.concatenate([f(b_ff1), f(b_ff2), b_t]))
    w1_p = _pack_km(f(W1), DT, FT).astype(bf)
    w2_p = _pack_km(f(W2), FT, DT).astype(bf)

    common = dict(
        wh=wh_p, wx=wx_p, bias=bias_gm,
        ident=np.eye(P).astype(bf),
        beta=_gm(f(lif_beta)), vth=_gm(f(lif_vth)),
        ln1g=f(ln1_g), ln1b=f(ln1_b), ln2g=f(ln2_g), ln2b=f(ln2_b),
        w1=w1_p, b1=_gm(f(b1)), w2=w2_p, b2=_gm(f(b2)),
    )
    in_maps = []
    for b in range(B):
        m = dict(common)
        m["x"] = np.ascontiguousarray(x[b])
        m["hx"] = np.ascontiguousarray(hx[b].reshape(KT, P).T)
        in_maps.append(m)

    nc = _get_program()
    res = run_bass_kernel_spmd(nc, in_maps, core_ids=list(range(B)),
                               trace=TRACE)
    last_exec_time_ns = res.exec_time_ns

    x_out = np.stack([res.results[b]["y_out"] for b in range(B)])
    h_final = np.stack(
        [res.results[b]["h_out"].T.reshape(U) for b in range(B)])
    return x_out, h_final


# revision 19
# speedup vs baseline: 1.0187x; 1.0006x over previous
"""Trainium2 Bass kernel for nn_AudioLiquidEmberBlock.

Computation (per batch element, B=8 sharded across 8 NeuronCores):
  normed = LN1(x)                                      # [T=1024, D=1024]
  scan over T with CfC cell (tanh/tanh/sigmoid gates, U=1536)
  fused with LIF spiking gate on the first D hidden channels
  x = x + gated; x = x + MLP(LN2(x))                   # MLP d_ff=4096
  returns (x, h_final)

Strategy:
  - Batch-parallel over 8 cores (batch 8 -> 1 sequence per core).
  - The x-contribution of the CfC gates (x_t @ W_x + b) is hoisted out of
    the scan into one big GEMM; only the h-recurrence (h @ W_h) runs in the
    sequential loop. Since TS == 1.0, the ta/tb gates fold into a single
    combined gate matrix, so the scan does 3 gate matmuls, not 4.
  - Scan uses gate-major layout: weights stationary on the PE
    ([128,128] bf16 tiles, FWL), hidden state streams as rhs columns.
  - All matmuls bf16 with fp32 PSUM accumulation; LN/elementwise fp32.
"""
import os
import sys

sys.path.insert(0, '/opt/trn_rl_repo')

import numpy as np
import ml_dtypes

import concourse.bass as bass
import concourse.tile as tile
from concourse import bacc, mybir
from concourse.bass import ds
from concourse.bass_utils import run_bass_kernel_spmd

P = 128
D = 1024          # n_embd
U = 1536          # CfC units
DFF = 4096        # MLP hidden
T = 1024          # sequence length
B = 8             # batch == number of cores
KT = U // P       # 12 k-tiles for the recurrence
MT = 3 * (U // P)  # 36 m-tiles (3 gates after ta/tb fold)
DT = D // P       # 8
FT = DFF // P     # 32
UNROLL = 64
HALF = KT // 2
NCH = 512         # T-chunk for the xw precompute GEMM
EPS = 1e-5
SLOPE = 5.0
TS = 1.0

f32 = mybir.dt.float32
bf16 = mybir.dt.bfloat16
AF = mybir.ActivationFunctionType
ET = mybir.EngineType

TRACE = bool(int(os.environ.get("KERNEL_TRACE", "0")))
last_exec_time_ns = None

_prog_cache = {}


def _layer_norm(nc, pool, x_sb, g_bc, b_bc, out_sb, tag):
    """LN over the free dim (size D) of x_sb [P, D] fp32 -> out_sb."""
    mu = pool.tile([P, 1], f32, tag=f"{tag}_mu", name=f"{tag}_mu")
    nc.vector.reduce_sum(mu[:], x_sb[:], axis=mybir.AxisListType.X)
    nc.vector.tensor_scalar_mul(mu[:], mu[:], 1.0 / D)
    xc = pool.tile([P, D], f32, tag=f"{tag}_xc", name=f"{tag}_xc")
    nc.vector.tensor_scalar_sub(xc[:], x_sb[:], mu[:])
    sq = pool.tile([P, D], f32, tag=f"{tag}_sq", name=f"{tag}_sq")
    var = pool.tile([P, 1], f32, tag=f"{tag}_var", name=f"{tag}_var")
    nc.scalar.activation(sq[:], xc[:], AF.Square, accum_out=var[:])
    rstd = pool.tile([P, 1], f32, tag=f"{tag}_rstd", name=f"{tag}_rstd")
    nc.vector.tensor_scalar(
        out=rstd[:], in0=var[:], scalar1=1.0 / D, scalar2=EPS,
        op0=mybir.AluOpType.mult, op1=mybir.AluOpType.add)
    nc.scalar.activation(rstd[:], rstd[:], AF.Sqrt)
    nc.vector.reciprocal(rstd[:], rstd[:])
    nc.vector.tensor_scalar_mul(xc[:], xc[:], rstd[:])
    nc.vector.tensor_mul(xc[:], xc[:], g_bc)
    nc.vector.tensor_add(out_sb[:], xc[:], b_bc)


def _build_program():
    nc = bacc.Bacc("TRN2", target_bir_lowering=False, debug=False,
                   enable_asserts=False, num_devices=B)

    x_d = nc.dram_tensor("x", [T, D], f32, kind="ExternalInput")
    hx_d = nc.dram_tensor("hx", [P, KT], f32, kind="ExternalInput")
    id_d = nc.dram_tensor("ident", [P, P], bf16, kind="ExternalInput")
    wh_d = nc.dram_tensor("wh", [P, KT, MT, P], bf16, kind="ExternalInput")
    wx_d = nc.dram_tensor("wx", [P, DT, MT, P], bf16, kind="ExternalInput")
    bias_d = nc.dram_tensor("bias", [P, MT], f32, kind="ExternalInput")
    beta_d = nc.dram_tensor("beta", [P, DT], f32, kind="ExternalInput")
    vth_d = nc.dram_tensor("vth", [P, DT], f32, kind="ExternalInput")
    ln1g_d = nc.dram_tensor("ln1g", [D], f32, kind="ExternalInput")
    ln1b_d = nc.dram_tensor("ln1b", [D], f32, kind="ExternalInput")
    ln2g_d = nc.dram_tensor("ln2g", [D], f32, kind="ExternalInput")
    ln2b_d = nc.dram_tensor("ln2b", [D], f32, kind="ExternalInput")
    w1_d = nc.dram_tensor("w1", [P, DT, FT, P], bf16, kind="ExternalInput")
    b1_d = nc.dram_tensor("b1", [P, FT], f32, kind="ExternalInput")
    w2_d = nc.dram_tensor("w2", [P, FT, DT, P], bf16, kind="ExternalInput")
    b2_d = nc.dram_tensor("b2", [P, DT], f32, kind="ExternalInput")

    y_out = nc.dram_tensor("y_out", [T, D], f32, kind="ExternalOutput")
    h_out = nc.dram_tensor("h_out", [P, KT], f32, kind="ExternalOutput")

    # DRAM scratch
    nrm_d = nc.dram_tensor("nrm_scratch", [T, D], bf16)
    xw_d = nc.dram_tensor("xw_scratch", [P, T, MT], bf16)
    gd_d = nc.dram_tensor("gd_scratch", [D, T], bf16)
    h2_d = nc.dram_tensor("h2_scratch", [T, D], bf16)
    o2_d = nc.dram_tensor("o2_scratch", [D, T], bf16)

    with tile.TileContext(nc) as tc:
        if True:
            # ---------------- Phase 1: LN1, write normed (bf16) ----------
            with (tc.tile_pool(name="ln1", bufs=2) as lp,
                  tc.tile_pool(name="lnc", bufs=1) as lc):
                ln1g_sb = lc.tile([P, D], f32, name="ln1g_sb")
                nc.sync.dma_start(ln1g_sb[:], ln1g_d[None, :].to_broadcast([P, D]))
                ln1b_sb = lc.tile([P, D], f32, name="ln1b_sb")
                nc.sync.dma_start(ln1b_sb[:], ln1b_d[None, :].to_broadcast([P, D]))
                g_bc = ln1g_sb[:]
                b_bc = ln1b_sb[:]
                for tt in range(DT):
                    xt = lp.tile([P, D], f32, tag="xt", name="xt")
                    nc.sync.dma_start(xt[:], x_d[tt * P:(tt + 1) * P, :])
                    nrmb = lp.tile([P, D], bf16, tag="nrmb", name="nrmb")
                    _layer_norm(nc, lp, xt, g_bc, b_bc, nrmb, "ln1")
                    nc.sync.dma_start(nrm_d[tt * P:(tt + 1) * P, :], nrmb[:])

            # ---------------- Phase 2+3: xw = normed @ Wx + b ------------
            with (tc.tile_pool(name="gm1w", bufs=1) as gw,
                  tc.tile_pool(name="gm1s", bufs=2) as gs,
                  tc.tile_pool(name="gm1p", bufs=4, space="PSUM") as gp):
                nrmT_sb = gw.tile([P, DT, T], bf16, name="nrmT_sb")  # 2 MB
                for dt_ in range(DT):
                    nc.sync.dma_start_transpose(
                        nrmT_sb[:, dt_, :], nrm_d[:, dt_ * P:(dt_ + 1) * P])
                wx_sb = gw.tile([P, DT, MT, P], bf16, name="wx_sb")  # 9.4 MB
                nc.sync.dma_start(wx_sb[:], wx_d[:])
                bias_sb = gw.tile([P, MT], f32, name="bias_sb")
                nc.sync.dma_start(bias_sb[:], bias_d[:])
                for n in range(T // NCH):
                    stg = gs.tile([P, NCH, MT], bf16, tag="stg", name="stg")
                    for m in range(MT):
                        psA = gp.tile([P, NCH], f32, tag="psA", name="psA")
                        for k in range(DT):
                            nc.tensor.matmul(
                                psA[:], wx_sb[:, k, m, :],
                                nrmT_sb[:, k, n * NCH:(n + 1) * NCH],
                                start=(k == 0), stop=(k == DT - 1))
                        nc.scalar.activation(stg[:, :, m], psA[:], AF.Identity,
                                             bias=bias_sb[:, m:m + 1])
                    nc.sync.dma_start(xw_d[:, n * NCH:(n + 1) * NCH, :], stg[:])

            # ---------------- Phase 4: the scan --------------------------
            with (tc.tile_pool(name="scw", bufs=1) as sw,
                  tc.tile_pool(name="scs", bufs=1) as st,
                  tc.tile_pool(name="scx", bufs=3) as sx,
                  tc.tile_pool(name="scwk", bufs=3) as wk,
                  tc.tile_pool(name="scps", bufs=2, space="PSUM") as pp):
                wh_sb = sw.tile([P, KT, MT, P], bf16, name="wh_sb")  # 14.2 MB
                nc.sync.dma_start(wh_sb[:], wh_d[:])
                id_sb = sw.tile([P, P], bf16, name="id_sb")
                nc.sync.dma_start(id_sb[:], id_d[:])
                beta_sb = st.tile([P, DT], f32, name="beta_sb")
                nc.sync.dma_start(beta_sb[:], beta_d[:])
                nc.scalar.activation(beta_sb[:], beta_sb[:], AF.Sigmoid)
                vth_sb = st.tile([P, DT], f32, name="vth_sb")
                nc.sync.dma_start(vth_sb[:], vth_d[:])
                gatedT = st.tile([P, DT, T], bf16, name="gatedT")   # 2 MB
                mem = st.tile([P, DT], f32, name="mem")
                nc.any.memset(mem[:], 0.0)
                hT = []
                for i in range(4):
                    ha = st.tile([P, HALF], bf16, name=f"h{i}a")
                    hb = st.tile([P, HALF], bf16, name=f"h{i}b")
                    hT.append((ha, hb))
                hf32 = st.tile([P, KT], f32, name="hf32")
                hx_sb = st.tile([P, KT], f32, name="hx_sb")
                nc.sync.dma_start(hx_sb[:], hx_d[:])
                nc.vector.tensor_copy(hT[0][0][:], hx_sb[:, 0:HALF])
                nc.vector.tensor_copy(hT[0][1][:], hx_sb[:, HALF:KT])
                nc.vector.tensor_copy(hf32[:], hx_sb[:])

                with tc.For_i(0, T, UNROLL, hint_engines=(ET.PE,),
                              staggered_reset=True) as t0:
                    xwc = sx.tile([P, UNROLL, MT], bf16, name="xwc")
                    nc.sync.dma_start(xwc[:], xw_d[:, ds(t0, UNROLL), :])

                    def init_psums(u):
                        pst = pp.tile([P, KT], f32, tag="pst", name="pst",
                                      padded_shape=[P, 512])
                        ps2 = pp.tile([P, KT], f32, tag="ps2", name="ps2",
                                      padded_shape=[P, 512])
                        ps1a = pp.tile([P, HALF], f32, tag="ps1a", name="ps1a",
                                       padded_shape=[P, 512])
                        ps1b = pp.tile([P, HALF], f32, tag="ps1b", name="ps1b",
                                       padded_shape=[P, 512])
                        nc.tensor.matmul(pst[:], id_sb[:],
                                         xwc[:, u, 2 * KT:3 * KT],
                                         start=True, stop=False)
                        nc.tensor.matmul(ps2[:], id_sb[:],
                                         xwc[:, u, KT:2 * KT],
                                         start=True, stop=False)
                        nc.tensor.matmul(ps1a[:], id_sb[:], xwc[:, u, 0:HALF],
                                         start=True, stop=False)
                        nc.tensor.matmul(ps1b[:], id_sb[:], xwc[:, u, HALF:KT],
                                         start=True, stop=False)
                        return pst, ps2, ps1a, ps1b

                    psums = init_psums(0)
                    for u in range(UNROLL):
                        curA, curB = hT[u % 4]
                        nxtA, nxtB = hT[(u + 1) % 4]

                        def rhs_col(k):
                            return curA[:, k:k + 1] if k < HALF else \
                                curB[:, k - HALF:k - HALF + 1]

                        pst, ps2, ps1a, ps1b = psums
                        # tc gate: K-OUTER (consumes cur column-by-column)
                        for k in range(KT):
                            for mi in range(KT):
                                nc.tensor.matmul(
                                    pst[:, mi:mi + 1],
                                    wh_sb[:, k, 2 * KT + mi, :],
                                    rhs_col(k), start=False,
                                    stop=(k == KT - 1), skip_group_check=True)
                        sg = wk.tile([P, KT], f32, tag="sg", name="sg")
                        nc.scalar.activation(sg[:], pst[:], AF.Sigmoid)
                        oms = wk.tile([P, KT], f32, tag="oms", name="oms")
                        nc.vector.tensor_scalar(
                            out=oms[:], in0=sg[:], scalar1=-1.0, scalar2=1.0,
                            op0=mybir.AluOpType.mult, op1=mybir.AluOpType.add)
                        if u + 1 < UNROLL:
                            psums = init_psums(u + 1)
                        # ff2 gate: m-outer
                        for mi in range(KT):
                            for k in range(KT):
                                nc.tensor.matmul(
                                    ps2[:, mi:mi + 1], wh_sb[:, k, KT + mi, :],
                                    rhs_col(k), start=False,
                                    stop=(k == KT - 1))
                        ff2 = wk.tile([P, KT], f32, tag="ff2", name="ff2")
                        nc.scalar.activation(ff2[:], ps2[:], AF.Tanh)
                        cc = wk.tile([P, KT], f32, tag="cc", name="cc")
                        nc.vector.tensor_mul(cc[:], sg[:], ff2[:])
                        # ff1 gate: m-outer, tail in column halves
                        ff1 = wk.tile([P, KT], f32, tag="ff1", name="ff1")
                        t1 = wk.tile([P, KT], f32, tag="t1", name="t1")
                        for half, (psh, nxth) in enumerate(((ps1a, nxtA),
                                                            (ps1b, nxtB))):
                            sl = slice(half * HALF, (half + 1) * HALF)
                            for mj in range(HALF):
                                mi = half * HALF + mj
                                for k in range(KT):
                                    nc.tensor.matmul(
                                        psh[:, mj:mj + 1], wh_sb[:, k, mi, :],
                                        rhs_col(k), start=False,
                                        stop=(k == KT - 1))
                            nc.scalar.activation(ff1[:, sl], psh[:], AF.Tanh)
                            nc.vector.tensor_mul(t1[:, sl], ff1[:, sl],
                                                 oms[:, sl])
                            nc.vector.tensor_add(nxth[:], t1[:, sl], cc[:, sl])
                        # off path: fp32 h for LIF + h_final
                        nc.vector.tensor_add(hf32[:], t1[:], cc[:])
                        # LIF membrane update on the first DT tiles
                        m1 = wk.tile([P, DT], f32, tag="m1", name="m1")
                        nc.vector.tensor_mul(m1[:], mem[:], beta_sb[:])
                        nc.vector.tensor_add(m1[:], m1[:], hf32[:, 0:DT])
                        spin = wk.tile([P, DT], f32, tag="spin", name="spin")
                        nc.vector.tensor_sub(spin[:], m1[:], vth_sb[:])
                        spk = wk.tile([P, DT], f32, tag="spk", name="spk")
                        nc.scalar.activation(spk[:], spin[:], AF.Sigmoid,
                                             scale=SLOPE)
                        sv = wk.tile([P, DT], f32, tag="sv", name="sv")
                        nc.vector.tensor_mul(sv[:], spk[:], vth_sb[:])
                        nc.vector.tensor_sub(mem[:], m1[:], sv[:])
                        nc.vector.tensor_mul(gatedT[:, :, ds(t0 + u, 1)],
                                             hf32[:, 0:DT, None],
                                             spk[:, :, None])
                nc.sync.dma_start(h_out[:], hf32[:])
                nc.sync.dma_start(
                    gd_d.rearrange("(dt p) t -> p dt t", p=P), gatedT[:])

            # ---------------- Phase 5: residual + LN2 --------------------
            pers_cm = tc.tile_pool(name="persist", bufs=1)
            pers = pers_cm.__enter__()
            x2_sb = pers.tile([P, DT, D], f32, name="x2_sb")       # 4 MB
            h2T_sb = pers.tile([P, DT, T], bf16, name="h2T_sb")    # 2 MB
            mlp1T_sb = pers.tile([P, FT, T], bf16, name="mlp1T_sb")  # 8 MB
            with (tc.tile_pool(name="ln2", bufs=2) as lp2,
                  tc.tile_pool(name="lnc2", bufs=1) as lc2):
                ln2g_sb = lc2.tile([P, D], f32, name="ln2g_sb")
                nc.sync.dma_start(ln2g_sb[:], ln2g_d[None, :].to_broadcast([P, D]))
                ln2b_sb = lc2.tile([P, D], f32, name="ln2b_sb")
                nc.sync.dma_start(ln2b_sb[:], ln2b_d[None, :].to_broadcast([P, D]))
                g2_bc = ln2g_sb[:]
                b2_bc = ln2b_sb[:]
                for tt in range(DT):
                    gtok = lp2.tile([P, D], bf16, tag="gtok", name="gtok")
                    nc.sync.dma_start_transpose(
                        gtok[:], gd_d[:, tt * P:(tt + 1) * P])
                    xt2 = lp2.tile([P, D], f32, tag="xt2", name="xt2")
                    nc.sync.dma_start(xt2[:], x_d[tt * P:(tt + 1) * P, :])
                    nc.vector.tensor_add(x2_sb[:, tt, :], xt2[:], gtok[:])
                    h2b = lp2.tile([P, D], bf16, tag="h2b", name="h2b")
                    _layer_norm(nc, lp2, x2_sb[:, tt, :], g2_bc, b2_bc, h2b,
                                "ln2")
                    nc.sync.dma_start(h2_d[tt * P:(tt + 1) * P, :], h2b[:])
                for dt_ in range(DT):
                    nc.sync.dma_start_transpose(
                        h2T_sb[:, dt_, :], h2_d[:, dt_ * P:(dt_ + 1) * P])

            # ---------------- Phase 6: MLP GEMM1 + gelu ------------------
            with (tc.tile_pool(name="gm2w", bufs=1) as g2w,
                  tc.tile_pool(name="gm2p", bufs=4, space="PSUM") as g2p):
                w1_sb = g2w.tile([P, DT, FT, P], bf16, name="w1_sb")  # 8 MB
                nc.sync.dma_start(w1_sb[:], w1_d[:])
                b1_sb = g2w.tile([P, FT], f32, name="b1_sb")
                nc.sync.dma_start(b1_sb[:], b1_d[:])
                for m in range(FT):
                    for n2 in range(2):
                        psB = g2p.tile([P, 512], f32, tag="psB", name="psB")
                        for k in range(DT):
                            nc.tensor.matmul(
                                psB[:], w1_sb[:, k, m, :],
                                h2T_sb[:, k, n2 * 512:(n2 + 1) * 512],
                                start=(k == 0), stop=(k == DT - 1))
                        nc.scalar.activation(
                            mlp1T_sb[:, m, n2 * 512:(n2 + 1) * 512], psB[:],
                            AF.Gelu_apprx_tanh, bias=b1_sb[:, m:m + 1])

            # ---------------- Phase 7: MLP GEMM2 + final residual --------
            with (tc.tile_pool(name="gm3w", bufs=1) as g3w,
                  tc.tile_pool(name="gm3t", bufs=2) as g3t,
                  tc.tile_pool(name="gm3p", bufs=4, space="PSUM") as g3p):
                w2_sb = g3w.tile([P, FT, DT, P], bf16, name="w2_sb")  # 8 MB
                nc.sync.dma_start(w2_sb[:], w2_d[:])
                b2_sb = g3w.tile([P, DT], f32, name="b2_sb")
                nc.sync.dma_start(b2_sb[:], b2_d[:])
                o2v = o2_d.rearrange("(dt p) t -> p dt t", p=P)
                for m in range(DT):
                    for n2 in range(2):
                        psC = g3p.tile([P, 512], f32, tag="psC", name="psC")
                        for k in range(FT):
                            nc.tensor.matmul(
                                psC[:], w2_sb[:, k, m, :],
                                mlp1T_sb[:, k, n2 * 512:(n2 + 1) * 512],
                                start=(k == 0), stop=(k == FT - 1))
                        o2s = g3t.tile([P, 512], bf16, tag="o2s", name="o2s")
                        nc.scalar.activation(o2s[:], psC[:], AF.Identity,
                                             bias=b2_sb[:, m:m + 1])
                        nc.sync.dma_start(
                            o2v[:, m, n2 * 512:(n2 + 1) * 512], o2s[:])
                for tt in range(DT):
                    otok = g3t.tile([P, D], bf16, tag="otok", name="otok")
                    nc.sync.dma_start_transpose(
                        otok[:], o2_d[:, tt * P:(tt + 1) * P])
                    yt = g3t.tile([P, D], f32, tag="yt", name="yt")
                    nc.vector.tensor_add(yt[:], x2_sb[:, tt, :], otok[:])
                    nc.sync.dma_start(y_out[tt * P:(tt + 1) * P, :], yt[:])
            pers_cm.__exit__(None, None, None)

    nc.compile()
    return nc


def _get_program():
    if "nc" not in _prog_cache:
        _prog_cache["nc"] = _build_program()
    return _prog_cache["nc"]


def _pack_km(w, kt, mt):
    """[kt*128, mt*128] -> [128, kt, mt, 128]"""
    return np.ascontiguousarray(
        w.reshape(kt, P, mt, P).transpose(1, 0, 2, 3))


def _gm(v):
    """[n*128] channel vector -> gate-major [128, n]"""
    n = v.shape[0] // P
    return np.ascontiguousarray(v.reshape(n, P).T)


def kernel(x, hx, ln1_g, ln1_b, W_ff1, b_ff1, W_ff2, b_ff2, W_ta, b_ta,
           W_tb, b_tb, lif_beta, lif_vth, ln2_g, ln2_b, W1, b1, W2, b2):
    global last_exec_time_ns
    x = np.asarray(x, np.float32)
    hx = np.asarray(hx, np.float32)

    f = lambda a: np.asarray(a, np.float32)
    W_t = TS * f(W_ta) + f(W_tb)
    b_t = TS * f(b_ta) + f(b_tb)
    Wall = np.concatenate([f(W_ff1), f(W_ff2), W_t], axis=1)  # [2560, 4608]
    bf = ml_dtypes.bfloat16
    wx_p = _pack_km(Wall[:D], DT, MT).astype(bf)
    wh_p = _pack_km(Wall[D:], KT, MT).astype(bf)
    bias_gm = _gm(np.concatenate([f(b_ff1), f(b_ff2), b_t]))
    w1_p = _pack_km(f(W1), DT, FT).astype(bf)
    w2_p = _pack_km(f(W2), FT, DT).astype(bf)

    common = dict(
        wh=wh_p, wx=wx_p, bias=bias_gm,
        ident=np.eye(P).astype(bf),
        beta=_gm(f(lif_beta)), vth=_gm(f(lif_vth)),
        ln1g=f(ln1_g), ln1b=f(ln1_b), ln2g=f(ln2_g), ln2b=f(ln2_b),
        w1=w1_p, b1=_gm(f(b1)), w2=w2_p, b2=_gm(f(b2)),
    )
    in_maps = []
    for b in range(B):
        m = dict(common)
        m["x"] = np.ascontiguousarray(x[b])
        m["hx"] = np.ascontiguousarray(hx[b].reshape(KT, P).T)
        in_maps.append(m)

    nc = _get_program()
    res = run_bass_kernel_spmd(nc, in_maps, core_ids=list(range(B)),
                               trace=TRACE)
    last_exec_time_ns = res.exec_time_ns

    x_out = np.stack([res.results[b]["y_out"] for b in range(B)])
    h_final = np.stack(
        [res.results[b]["h_out"].T.reshape(U) for b in range(B)])
    return x_out, h_final
